# revision 44
# baseline (speedup 1.0000x reference)
"""Trainium2 Bass kernel for nn_AttenMlpFinal (attention-MLP pooling).

Reference (per batch row b):
    xx[m]  = concat(q[b], k[b,m])                  # [2D]
    h      = relu(xx @ W1^T)                       # [M, H]
    scores = h @ W2^T                              # [M]
    attn   = softmax(scores over m)
    out[b] = sum_m attn[m] * v[b,m]                # [D]

Strategy (pure data parallel over bsz across 8 cores; bf16 matmul inputs):
  Fold |W2_h| into W1 row h (relu scale-invariance), permute hidden units
  by W2 sign into [neg (Q, "ACT group") | pos (L, "DVE group")]:
    scores[b,m] = sum_pos relu(P+K) - sum_neg relu(P+K)   (|W2|-scaled)
  Both sums are plain relu-accumulations over PSUM preactivations, split
  across the two engines that can read PSUM:
    PE  (bf16, FWL): per (block,m) two N=512 matmuls build P+K in one
        PSUM bank (q-fold start=True, k start=False accumulate); v-sum
        via identity-stationary accumulating matmuls over e-scaled v.
    ACT: relu+accum on the neg group -> sc_a; exp; final 1/denom scale.
    DVE: tensor_scalar max(.,0)+accum on the pos group -> sc_x; softmax
         combine; v*e in one stride-0-broadcast multiply per block.
  softmax over m=8 without max subtraction (scores are O(1)); q-only
  linear terms are constant over m and cancel in softmax.
  k and q ship pre-transposed and group-major (one contiguous
  8KB-per-partition DMA descriptor per group) so there are no on-chip
  transposes and no DRAM staging copies.
"""

import sys

sys.path.insert(0, "/opt/trn_rl_repo")

from contextlib import ExitStack

import numpy as np
import ml_dtypes

import concourse.bass as bass
import concourse.tile as tile
from concourse import bacc, mybir
from concourse.bass_utils import run_bass_kernel_spmd


F32 = mybir.dt.float32
BF16 = mybir.dt.bfloat16
ALU = mybir.AluOpType
ACTF = mybir.ActivationFunctionType

N_CORES = 8
BSZ, M, D, H = 32768, 8, 128, 512
B = BSZ // N_CORES  # rows per core

GROUP = 4  # b-blocks per v-sum matmul group (psum bank = 4*128 fp32 cols)

BF = ml_dtypes.bfloat16


def build_nc(b_per_core: int, Q: int):
    """Q = ACT (neg) group size; L = H - Q = DVE (pos) group size."""
    L = H - Q
    nb = b_per_core // 128
    ngroups = nb // GROUP
    assert nb % GROUP == 0

    nc = bacc.Bacc("TRN2", target_bir_lowering=False, debug=False)

    # group-major host layouts: every SBUF load is one contiguous
    # 8KB-per-partition descriptor
    kT = nc.declare_dram_parameter(
        "kT", [ngroups, D, M * GROUP * 128], BF16, isOutput=False
    )
    qT = nc.declare_dram_parameter("qT", [D, b_per_core], BF16, isOutput=False)
    v = nc.declare_dram_parameter(
        "v", [ngroups, 128, GROUP * M * D], BF16, isOutput=False
    )
    wk = nc.declare_dram_parameter("wk", [D, H], BF16, isOutput=False)
    wq = nc.declare_dram_parameter("wq", [D, H], BF16, isOutput=False)
    ident = nc.declare_dram_parameter("ident", [128, 128], BF16, isOutput=False)
    # partition-major per group; host reassembles to [B, D]
    out = nc.declare_dram_parameter(
        "out", [ngroups, 128, GROUP * D], F32, isOutput=True
    )

    with tile.TileContext(nc) as tc, ExitStack() as ctx:
        consts = ctx.enter_context(tc.tile_pool(name="consts", bufs=1))
        qpool = ctx.enter_context(tc.tile_pool(name="qpool", bufs=1))
        kpool = ctx.enter_context(tc.tile_pool(name="kpool", bufs=3))
        vpool = ctx.enter_context(tc.tile_pool(name="vpool", bufs=3))
        scr = ctx.enter_context(tc.tile_pool(name="scr", bufs=6))
        smax = ctx.enter_context(tc.tile_pool(name="smax", bufs=2 * GROUP + 2))
        vsc = ctx.enter_context(tc.tile_pool(name="vsc", bufs=2))
        outp = ctx.enter_context(tc.tile_pool(name="outp", bufs=2))

        ps_a = ctx.enter_context(tc.tile_pool(name="ps_a", bufs=6, space="PSUM"))
        ps_vo = ctx.enter_context(tc.tile_pool(name="ps_vo", bufs=1, space="PSUM"))
        ps_sc = ctx.enter_context(tc.tile_pool(name="ps_sc", bufs=1, space="PSUM"))

        # ---- constants ----
        wk_sb = consts.tile([D, H], BF16, tag="wk")
        nc.sync.dma_start(out=wk_sb[:], in_=wk[:])
        wq_sb = consts.tile([D, H], BF16, tag="wq")
        nc.sync.dma_start(out=wq_sb[:], in_=wq[:])
        id_sb = consts.tile([128, 128], BF16, tag="ident")
        nc.sync.dma_start(out=id_sb[:], in_=ident[:])

        # Direct loads from external buffers (group-major contiguous layout;
        # no internal-DRAM staging, no extra HBM traffic).
        qT_sb = qpool.tile([D, b_per_core], BF16)
        nc.sync.dma_start(out=qT_sb[:], in_=qT[:])

        def emit_vsum(g_prev, vscaled_prev, recs_prev):
            # v-sum via identity-stationary accumulating matmuls; emitted one
            # group late so these PE ops (which depend on the previous
            # group's last DVE v-scales) never head-of-line-block the PE
            # queue: by emission time their inputs are long since ready.
            # v was scaled by unnormalized e; 1/denom lands here via DVE.
            vo_ps = ps_vo.tile([128, GROUP * 128], F32)
            for m in range(M):
                nc.tensor.matmul(
                    vo_ps[:],
                    id_sb[:],
                    vscaled_prev[:, m, :, :],
                    start=(m == 0),
                    stop=(m == M - 1),
                )
            out_sb = outp.tile([128, GROUP, 128], F32)
            for j in range(GROUP):
                nc.vector.tensor_scalar_mul(
                    out_sb[:, j, :],
                    vo_ps[:, j * 128 : (j + 1) * 128],
                    recs_prev[j][:],
                )
            nc.sync.dma_start(
                out=out[g_prev, :, :], in_=out_sb[:, :, :]
            )

        pending_vsum = None  # (gb, vscaled, recs) of the previous group

        for g in range(ngroups):
            gb = g * GROUP * 128  # first b row of this group

            kT_sb = kpool.tile([D, M, GROUP * 128], BF16)
            nc.sync.dma_start(out=kT_sb[:], in_=kT[g, :, :])
            v_sb = vpool.tile([128, GROUP, M, D], BF16)
            nc.sync.dma_start(out=v_sb[:, :, :, :], in_=v[g, :, :])

            vscaled = vsc.tile([128, M, GROUP, 128], BF16, tag="vs", name="vs")
            recs = []

            for j in range(GROUP):
                qsl = qT_sb[:, gb + j * 128 : gb + (j + 1) * 128]

                sc_a = ps_sc.tile([128, M], F32, tag="sc_a")
                sc_x = smax.tile([128, M], F32, tag="sc_x")

                # ---- per-m main work ----
                for m in range(M):
                    ksl = kT_sb[:, m, j * 128 : (j + 1) * 128]
                    a_ps = ps_a.tile([128, H], F32)
                    # full preactivation P+K for all H cols
                    nc.tensor.matmul(
                        a_ps[:], qsl, wq_sb[:], start=True, stop=False
                    )
                    nc.tensor.matmul(
                        a_ps[:], ksl, wk_sb[:], start=False, stop=True
                    )
                    t_a = scr.tile([128, Q], BF16, tag="scra")
                    nc.scalar.activation(
                        t_a[:], a_ps[:, 0:Q], ACTF.Relu,
                        accum_out=sc_a[:, m : m + 1],
                    )
                    t_x = scr.tile([128, L], BF16, tag="scrx")
                    nc.vector.tensor_scalar(
                        t_x[:], a_ps[:, Q:H], 0.0, 0.0,
                        op0=ALU.max,
                        op1=ALU.add,
                        accum_out=sc_x[:, m : m + 1],
                    )

                # ---- scores = sc_x - sc_a ; softmax over m ----
                scores = smax.tile([128, M], F32, tag="scores")
                nc.vector.tensor_sub(scores[:], sc_x[:], sc_a[:])
                e_sb = smax.tile([128, M], F32, tag="e")
                nc.scalar.activation(e_sb[:], scores[:], ACTF.Exp)
                denom = smax.tile([128, 1], F32, tag="denom")
                nc.vector.tensor_reduce(
                    denom[:], e_sb[:], mybir.AxisListType.X, ALU.add
                )
                rec = smax.tile([128, 1], F32, tag="recip")
                nc.vector.reciprocal(rec[:], denom[:])
                recs.append(rec)

                # ---- v * e_m, all 8 m in one broadcast op (normalization
                # by 1/denom is applied later at the output stage) ----
                e_b = e_sb[:].unsqueeze(2).broadcast_to([128, M, 128])
                nc.vector.scalar_tensor_tensor(
                    out=vscaled[:, :, j, :],
                    in0=v_sb[:, j, :, :],
                    scalar=0.0,
                    in1=e_b,
                    op0=ALU.bypass,
                    op1=ALU.mult,
                )

            if pending_vsum is not None:
                emit_vsum(*pending_vsum)
            pending_vsum = (g, vscaled, recs)

        emit_vsum(*pending_vsum)

    nc.compile()
    return nc


def host_prep(q_vec, k_vec, v_vec, W1, W2, b_per_core):
    """Host-side resharding + weight preprocessing (numpy only)."""
    W1 = np.asarray(W1, dtype=np.float32)
    w2 = np.asarray(W2, dtype=np.float32).reshape(-1)  # [H]

    neg = w2 < 0
    order = np.concatenate([np.where(neg)[0], np.where(~neg)[0]])
    Q = int(neg.sum())

    Ws = (np.abs(w2)[:, None] * W1)[order]  # [H, 2D] |W2|-folded, permuted
    Wsq, Wsk = Ws[:, :D], Ws[:, D:]

    wk_b = np.ascontiguousarray(Wsk.T).astype(BF)  # [D, H]
    wq_b = np.ascontiguousarray(Wsq.T).astype(BF)  # [D, H]
    ident = np.eye(128, dtype=np.float32).astype(BF)

    GB = GROUP * 128
    ngroups = b_per_core // GB
    in_maps = []
    n_cores = len(q_vec) // b_per_core
    for c in range(n_cores):
        sl = slice(c * b_per_core, (c + 1) * b_per_core)
        k_sh = np.asarray(k_vec[sl], dtype=np.float32)
        q_sh = np.asarray(q_vec[sl], dtype=np.float32)
        v_sh = np.asarray(v_vec[sl], dtype=np.float32)
        # group-major: kT [g, D, M, 512cols], v [g, p, GROUP, M*D]
        kT_h = k_sh.reshape(ngroups, GB, M, D).transpose(0, 3, 2, 1)
        v_h = v_sh.reshape(ngroups, GROUP, 128, M * D).transpose(0, 2, 1, 3)
        in_maps.append(
            {
                "kT": np.ascontiguousarray(kT_h).reshape(ngroups, D, M * GB).astype(BF),
                "qT": np.ascontiguousarray(q_sh.T).astype(BF),
                "v": np.ascontiguousarray(v_h).reshape(ngroups, 128, GROUP * M * D).astype(BF),
                "wk": wk_b,
                "wq": wq_b,
                "ident": ident,
            }
        )
    return in_maps, Q


_NC_CACHE = {}


def kernel(q_vec, k_vec, v_vec, W1, W2):
    in_maps, Q = host_prep(q_vec, k_vec, v_vec, W1, W2, B)
    key = (B, Q)
    if key not in _NC_CACHE:
        _NC_CACHE[key] = build_nc(B, Q)
    nc = _NC_CACHE[key]
    res = run_bass_kernel_spmd(nc, in_maps, list(range(N_CORES)))
    ngroups = B // (GROUP * 128)
    outs = []
    for c in range(N_CORES):
        o = res.results[c]["out"]  # [ngroups, 128, GROUP*D] partition-major
        o = o.reshape(ngroups, 128, GROUP, D).transpose(0, 2, 1, 3).reshape(B, D)
        outs.append(o)
    return np.ascontiguousarray(np.concatenate(outs, axis=0), dtype=np.float32)


if __name__ == "__main__":
    rng = np.random.default_rng(0)
    q = rng.standard_normal((BSZ, D), dtype=np.float32)
    k = rng.standard_normal((BSZ, M, D), dtype=np.float32)
    v = rng.standard_normal((BSZ, M, D), dtype=np.float32)
    W1 = (rng.standard_normal((H, 2 * D)) / np.sqrt(2 * D)).astype(np.float32)
    W2 = (rng.standard_normal((1, H)) / np.sqrt(H)).astype(np.float32)
    o = kernel(q, k, v, W1, W2)
    print(o.shape, o.dtype)


# revision 50
# speedup vs baseline: 1.2265x; 1.2265x over previous
"""Trainium2 Bass kernel for nn_AttenMlpFinal (attention-MLP pooling).

Reference (per batch row b):
    xx[m]  = concat(q[b], k[b,m])                  # [2D]
    h      = relu(xx @ W1^T)                       # [M, H]
    scores = h @ W2^T                              # [M]
    attn   = softmax(scores over m)
    out[b] = sum_m attn[m] * v[b,m]                # [D]

Strategy (pure data parallel over bsz across 8 cores; bf16 matmul inputs):
  Fold |W2_h| into W1 row h (relu scale-invariance), permute hidden units
  by W2 sign into [neg (Q, "ACT group") | pos (L, "DVE group")]:
    scores[b,m] = sum_pos relu(P+K) - sum_neg relu(P+K)   (|W2|-scaled)
  Both sums are plain relu-accumulations over PSUM preactivations, split
  across the two engines that can read PSUM:
    PE  (bf16, FWL): per (block,m) two N=512 matmuls build P+K in one
        PSUM bank (q-fold start=True, k start=False accumulate); v-sum
        via identity-stationary accumulating matmuls over e-scaled v.
    ACT: relu+accum on the neg group -> sc_a; exp; final 1/denom scale.
    DVE: tensor_scalar max(.,0)+accum on the pos group -> sc_x; softmax
         combine; v*e in one stride-0-broadcast multiply per block.
  softmax over m=8 without max subtraction (scores are O(1)); q-only
  linear terms are constant over m and cancel in softmax.
  k and q ship pre-transposed and group-major (one contiguous
  8KB-per-partition DMA descriptor per group) so there are no on-chip
  transposes and no DRAM staging copies.
"""

import sys

sys.path.insert(0, "/opt/trn_rl_repo")

from contextlib import ExitStack

import numpy as np
import ml_dtypes

import concourse.bass as bass
import concourse.tile as tile
from concourse import bacc, mybir
from concourse.bass_utils import run_bass_kernel_spmd


F32 = mybir.dt.float32
BF16 = mybir.dt.bfloat16
ALU = mybir.AluOpType
ACTF = mybir.ActivationFunctionType

N_CORES = 8
BSZ, M, D, H = 32768, 8, 128, 512
B = BSZ // N_CORES  # rows per core

GROUP = 4  # b-blocks per v-sum matmul group (psum bank = 4*128 fp32 cols)

BF = ml_dtypes.bfloat16


def build_nc(b_per_core: int, Q: int):
    """Q = ACT (neg) group size; L = H - Q = DVE (pos) group size."""
    L = H - Q
    nb = b_per_core // 128
    ngroups = nb // GROUP
    assert nb % GROUP == 0

    nc = bacc.Bacc("TRN2", target_bir_lowering=False, debug=False)

    # group-major host layouts: every SBUF load is one contiguous
    # 8KB-per-partition descriptor
    kT = nc.declare_dram_parameter(
        "kT", [ngroups, D, M * GROUP * 128], BF16, isOutput=False
    )
    qT = nc.declare_dram_parameter("qT", [D, b_per_core], BF16, isOutput=False)
    v = nc.declare_dram_parameter(
        "v", [ngroups, 128, GROUP * M * D], BF16, isOutput=False
    )
    wk = nc.declare_dram_parameter("wk", [D, H], BF16, isOutput=False)
    wq = nc.declare_dram_parameter("wq", [D, H], BF16, isOutput=False)
    ident = nc.declare_dram_parameter("ident", [128, 128], BF16, isOutput=False)
    # partition-major per group; host reassembles to [B, D]
    out = nc.declare_dram_parameter(
        "out", [ngroups, 128, GROUP * D], F32, isOutput=True
    )

    with tile.TileContext(nc) as tc, ExitStack() as ctx:
        consts = ctx.enter_context(tc.tile_pool(name="consts", bufs=1))
        qpool = ctx.enter_context(tc.tile_pool(name="qpool", bufs=1))
        kpool = ctx.enter_context(tc.tile_pool(name="kpool", bufs=3))
        vpool = ctx.enter_context(tc.tile_pool(name="vpool", bufs=3))
        scr = ctx.enter_context(tc.tile_pool(name="scr", bufs=6))
        smax = ctx.enter_context(tc.tile_pool(name="smax", bufs=2 * GROUP + 2))
        vsc = ctx.enter_context(tc.tile_pool(name="vsc", bufs=2))
        outp = ctx.enter_context(tc.tile_pool(name="outp", bufs=2))

        ps_a = ctx.enter_context(tc.tile_pool(name="ps_a", bufs=6, space="PSUM"))
        ps_vo = ctx.enter_context(tc.tile_pool(name="ps_vo", bufs=2, space="PSUM"))

        # ---- constants ----
        wk_sb = consts.tile([D, H], BF16, tag="wk")
        nc.sync.dma_start(out=wk_sb[:], in_=wk[:])
        wq_sb = consts.tile([D, H], BF16, tag="wq")
        nc.sync.dma_start(out=wq_sb[:], in_=wq[:])
        id_sb = consts.tile([128, 128], BF16, tag="ident")
        nc.sync.dma_start(out=id_sb[:], in_=ident[:])

        # Direct loads from external buffers (group-major contiguous layout;
        # no internal-DRAM staging, no extra HBM traffic).
        qT_sb = qpool.tile([D, b_per_core], BF16)
        nc.sync.dma_start(out=qT_sb[:], in_=qT[:])

        def emit_vsum(g_prev, vscaled_prev, recs_prev):
            # v-sum via identity-stationary accumulating matmuls; emitted one
            # group late so these PE ops (which depend on the previous
            # group's last DVE v-scales) never head-of-line-block the PE
            # queue: by emission time their inputs are long since ready.
            # v was scaled by unnormalized e; 1/denom lands here via DVE.
            vo_ps = ps_vo.tile([128, GROUP * 128], F32)
            for m in range(M):
                nc.tensor.matmul(
                    vo_ps[:],
                    id_sb[:],
                    vscaled_prev[:, m, :, :],
                    start=(m == 0),
                    stop=(m == M - 1),
                )
            out_sb = outp.tile([128, GROUP, 128], F32)
            for j in range(GROUP):
                nc.vector.tensor_scalar_mul(
                    out_sb[:, j, :],
                    vo_ps[:, j * 128 : (j + 1) * 128],
                    recs_prev[j][:],
                )
            nc.sync.dma_start(
                out=out[g_prev, :, :], in_=out_sb[:, :, :]
            )

        def emit_tail(sc_a_p, sc_x_p, v_sb_p, vscaled_p, j_p, recs_p):
            # softmax + v*e for a block; emitted one block late so these
            # DVE ops (whose exp dependency hops through ACT) never
            # head-of-line-block the DVE queue for the next block's work.
            scores = smax.tile([128, M], F32, tag="scores")
            nc.vector.tensor_sub(scores[:], sc_x_p[:], sc_a_p[:])
            e_sb = smax.tile([128, M], F32, tag="e")
            nc.scalar.activation(e_sb[:], scores[:], ACTF.Exp)
            denom = smax.tile([128, 1], F32, tag="denom")
            nc.vector.tensor_reduce(
                denom[:], e_sb[:], mybir.AxisListType.X, ALU.add
            )
            rec = smax.tile([128, 1], F32, tag="recip")
            nc.vector.reciprocal(rec[:], denom[:])
            recs_p.append(rec)
            # v * e_m, all 8 m in one broadcast op (normalization by
            # 1/denom is applied later at the output stage)
            e_b = e_sb[:].unsqueeze(2).broadcast_to([128, M, 128])
            nc.vector.scalar_tensor_tensor(
                out=vscaled_p[:, :, j_p, :],
                in0=v_sb_p[:, j_p, :, :],
                scalar=0.0,
                in1=e_b,
                op0=ALU.bypass,
                op1=ALU.mult,
            )

        pending_vsum = None  # (g, vscaled, recs) of the previous group
        pending_tail = None  # deferred softmax tail of the previous block

        for g in range(ngroups):
            gb = g * GROUP * 128  # first b row of this group

            kT_sb = kpool.tile([D, M, GROUP * 128], BF16)
            nc.sync.dma_start(out=kT_sb[:], in_=kT[g, :, :])
            v_sb = vpool.tile([128, GROUP, M, D], BF16)
            nc.sync.dma_start(out=v_sb[:, :, :, :], in_=v[g, :, :])

            vscaled = vsc.tile([128, M, GROUP, 128], BF16, tag="vs", name="vs")
            recs = []

            for j in range(GROUP):
                qsl = qT_sb[:, gb + j * 128 : gb + (j + 1) * 128]

                sc_a = smax.tile([128, M], F32, tag="sc_a")
                sc_x = smax.tile([128, M], F32, tag="sc_x")

                # ---- per-m main work ----
                for m in range(M):
                    ksl = kT_sb[:, m, j * 128 : (j + 1) * 128]
                    a_ps = ps_a.tile([128, H], F32)
                    # full preactivation P+K for all H cols
                    nc.tensor.matmul(
                        a_ps[:], qsl, wq_sb[:], start=True, stop=False
                    )
                    nc.tensor.matmul(
                        a_ps[:], ksl, wk_sb[:], start=False, stop=True
                    )
                    t_a = scr.tile([128, Q], BF16, tag="scra")
                    nc.scalar.activation(
                        t_a[:], a_ps[:, 0:Q], ACTF.Relu,
                        accum_out=sc_a[:, m : m + 1],
                    )
                    t_x = scr.tile([128, L], BF16, tag="scrx")
                    nc.vector.tensor_scalar(
                        t_x[:], a_ps[:, Q:H], 0.0, 0.0,
                        op0=ALU.max,
                        op1=ALU.add,
                        accum_out=sc_x[:, m : m + 1],
                    )

                if pending_tail is not None:
                    emit_tail(*pending_tail)
                pending_tail = (sc_a, sc_x, v_sb, vscaled, j, recs)

            if pending_vsum is not None:
                emit_vsum(*pending_vsum)
            pending_vsum = (g, vscaled, recs)

        emit_tail(*pending_tail)
        emit_vsum(*pending_vsum)

    nc.compile()
    return nc


def host_prep(q_vec, k_vec, v_vec, W1, W2, b_per_core):
    """Host-side resharding + weight preprocessing (numpy only)."""
    W1 = np.asarray(W1, dtype=np.float32)
    w2 = np.asarray(W2, dtype=np.float32).reshape(-1)  # [H]

    neg = w2 < 0
    order = np.concatenate([np.where(neg)[0], np.where(~neg)[0]])
    Q = int(neg.sum())

    Ws = (np.abs(w2)[:, None] * W1)[order]  # [H, 2D] |W2|-folded, permuted
    Wsq, Wsk = Ws[:, :D], Ws[:, D:]

    wk_b = np.ascontiguousarray(Wsk.T).astype(BF)  # [D, H]
    wq_b = np.ascontiguousarray(Wsq.T).astype(BF)  # [D, H]
    ident = np.eye(128, dtype=np.float32).astype(BF)

    GB = GROUP * 128
    ngroups = b_per_core // GB
    in_maps = []
    n_cores = len(q_vec) // b_per_core
    for c in range(n_cores):
        sl = slice(c * b_per_core, (c + 1) * b_per_core)
        k_sh = np.asarray(k_vec[sl], dtype=np.float32)
        q_sh = np.asarray(q_vec[sl], dtype=np.float32)
        v_sh = np.asarray(v_vec[sl], dtype=np.float32)
        # group-major: kT [g, D, M, 512cols], v [g, p, GROUP, M*D]
        kT_h = k_sh.reshape(ngroups, GB, M, D).transpose(0, 3, 2, 1)
        v_h = v_sh.reshape(ngroups, GROUP, 128, M * D).transpose(0, 2, 1, 3)
        in_maps.append(
            {
                "kT": np.ascontiguousarray(kT_h).reshape(ngroups, D, M * GB).astype(BF),
                "qT": np.ascontiguousarray(q_sh.T).astype(BF),
                "v": np.ascontiguousarray(v_h).reshape(ngroups, 128, GROUP * M * D).astype(BF),
                "wk": wk_b,
                "wq": wq_b,
                "ident": ident,
            }
        )
    return in_maps, Q


_NC_CACHE = {}


def kernel(q_vec, k_vec, v_vec, W1, W2):
    in_maps, Q = host_prep(q_vec, k_vec, v_vec, W1, W2, B)
    key = (B, Q)
    if key not in _NC_CACHE:
        _NC_CACHE[key] = build_nc(B, Q)
    nc = _NC_CACHE[key]
    res = run_bass_kernel_spmd(nc, in_maps, list(range(N_CORES)))
    ngroups = B // (GROUP * 128)
    outs = []
    for c in range(N_CORES):
        o = res.results[c]["out"]  # [ngroups, 128, GROUP*D] partition-major
        o = o.reshape(ngroups, 128, GROUP, D).transpose(0, 2, 1, 3).reshape(B, D)
        outs.append(o)
    return np.ascontiguousarray(np.concatenate(outs, axis=0), dtype=np.float32)


if __name__ == "__main__":
    rng = np.random.default_rng(0)
    q = rng.standard_normal((BSZ, D), dtype=np.float32)
    k = rng.standard_normal((BSZ, M, D), dtype=np.float32)
    v = rng.standard_normal((BSZ, M, D), dtype=np.float32)
    W1 = (rng.standard_normal((H, 2 * D)) / np.sqrt(2 * D)).astype(np.float32)
    W2 = (rng.standard_normal((1, H)) / np.sqrt(H)).astype(np.float32)
    o = kernel(q, k, v, W1, W2)
    print(o.shape, o.dtype)
